# revision 1
# baseline (speedup 1.0000x reference)
"""Trainium2 Bass kernel for nn_MixedFeedFoward (DARTS-style mixed-architecture MLP).

Math: out = relu(x @ (m0*w0).T + bm0*b0) @ (m1*w1).T + bm1*b1
The DARTS masks are rank-structured.  With a = softmax(arch_embed),
b = softmax(arch_mlp), EMBED = (512,768,1024), RATIO = (2,3,4):

  s_e[h]     = sum_r b_r * [h < e*r]
  g_j[h]     = sum_{e_idx >= j} a_e * s_e[h]
  c_j        = sum_{e_idx >= j} a_e
  W0eff[h,d] = w0[h,d] * g_{blk(d)}[h]      blk(d): 0 for d<512, 1 for d<768, else 2
  bm0[h]     = g_0[h]
  W1eff[d,h] = w1[d,h] * g_{blk(d)}[h]
  bm1[d]     = c_{blk(d)}

g_j is constant on 256-aligned h segments, so masking reduces to 51 runtime
scalars (3 j's x 16 segments + 3 c's) computed on device from the arch inputs
via one tiny matmul against a constant 0/1 selection table (a 52nd all-ones
column yields the softmax normalizer, applied per-partition after broadcast).

Sharding: data-parallel over the 4096 tokens -> 512 tokens per core; every
core streams the full weights.  Engine assignment keeps the PE at its
216 ns/matmul issue roofline once real work starts:
  - Sync HWDGE queue: arch scalars first (low latency), then the w0/w1 weight
    stream (the only large consumer, ~290 GB/s sustained).
  - Scalar HWDGE queue: xT loads + bias vectors + output stores; Scalar engine
    compute does exp, most xT casts, relu, and the w1 masking.
  - DVE: w0 masking (f32 mask-mult with bf16 output, one op per 128-d chunk),
    part of the early w1 masking, and output accumulation.
Layer 0 runs k-major (4 concurrent PSUM chains per 512-row h-group) so its
HBM consumption rate matches DMA arrival; L1(pr0) consumes w1 pairs in DMA
arrival order (pair-major over 4-chain halves).  Phases alternate
L0(pr) / L1(pr), which tracks the DMA delivery envelope.

The first ~19 us are DMA-bound (preamble + x + first weight tiles + the mask
scalars); NWARM junk matmuls bridge that window so the HAM clock gate reaches
K=8/8 before the first real matmul and the PE never idles >3.4 us (which
would re-throttle it to 1.2 GHz).  NWARM also shifts the compile-time static
schedule discontinuously; 26 best by n=2 mean (142.5/143.3) vs 30 (n=4, 144.4)
(22 lands a pathological schedule at 170 us — re-measure before changing).
The hg0->hg1 boundary has an unavoidable DMA-bound idle (~4-5 us) which the
HAM punishes with ~2 half-clock windows regardless of how the idle is sliced
(the MID window triggers on idle DENSITY, not contiguous gaps — sentinel
matmuls that split the gap were tried and removed: no measurable win).
Run-to-run jitter is +/-2.5 us from the free-running HAM window phase; NWARM=26 measured {142.5, 143.3} us.
"""

import os

import numpy as np

import concourse.bass as bass
import concourse.mybir as mybir
from concourse import bacc
from concourse.bass_utils import run_bass_kernel_spmd
from concourse.tile import TileContext

N_CORES = 8
D = 1024          # embed dim
H = 4096          # expansion dim
T = 512           # tokens per core (4096 total / 8 cores)
P = 128
SEG = 256         # h-segment size on which g_j is constant
NSEG = H // SEG   # 16
EMBED = (512, 768, 1024)
RATIO = (2, 3, 4)

F32 = mybir.dt.float32
BF16 = mybir.dt.bfloat16
AF = mybir.ActivationFunctionType
ALU = mybir.AluOpType

NWARM = int(os.environ.get("BASS_NWARM", "26"))
FILL1 = int(os.environ.get("BASS_FILL1", "0"))    # junk MMs between hg0 and hg1
FILL2 = int(os.environ.get("BASS_FILL2", "0"))    # junk MMs before L1(pr0)


def _build_k2() -> np.ndarray:
    """Constant 0/1 selection table: G_flat[col] = sum_i E9[i] * K2[i, col]
    where E9[e*3+r] = exp(ae[e] + am[r]).
    cols 0..47: col = j*16 + seg -> [e_idx >= j] * [seg*SEG < e*r]
    cols 48..50: col = 48 + j   -> [e_idx >= j]   (since sum_r b_r = 1)
    col 51: all ones -> sum(E9), the softmax normalizer
    """
    k2 = np.zeros((9, 52), dtype=np.float32)
    for ie, e in enumerate(EMBED):
        for ir, r in enumerate(RATIO):
            i = ie * 3 + ir
            for j in range(3):
                if ie >= j:
                    for seg in range(NSEG):
                        if seg * SEG < e * r:
                            k2[i, j * 16 + seg] = 1.0
                    k2[i, 48 + j] = 1.0
            k2[i, 51] = 1.0
    return k2


_K2 = _build_k2()

# d-block of each 128-wide d-chunk (0..7): [0,512)->0, [512,768)->1, [768,1024)->2
_DBLK = [0, 0, 0, 0, 1, 1, 2, 2]


def _build_nc() -> bass.Bass:
    nc = bacc.Bacc("TRN2", target_bir_lowering=False, debug=False)

    xT_d = nc.dram_tensor("xT", [D, T], F32, kind="ExternalInput")
    w0T_d = nc.dram_tensor("w0T", [D, H], F32, kind="ExternalInput")
    w1T_d = nc.dram_tensor("w1T", [H, D], F32, kind="ExternalInput")
    b0r_d = nc.dram_tensor("b0r", [P, H // P], F32, kind="ExternalInput")
    b1r_d = nc.dram_tensor("b1r", [P, D // P], F32, kind="ExternalInput")
    # arch = [ae9 | am9 | K2] packed in one tensor: a single 216B-row DMA
    # completes several us sooner than three tiny-element DMAs
    arch_d = nc.dram_tensor("arch", [9, 54], F32, kind="ExternalInput")
    out_d = nc.dram_tensor("outT", [D, T], F32, kind="ExternalOutput")

    with TileContext(nc) as tc:
        with (
            tc.tile_pool(name="const", bufs=1) as const,
            tc.tile_pool(name="w0f", bufs=8) as w0f_pool,
            tc.tile_pool(name="xfp", bufs=1) as xf_pool,
            tc.tile_pool(name="w0p", bufs=16) as w0_pool,
            tc.tile_pool(name="w1f", bufs=7) as w1f_pool,
            tc.tile_pool(name="w1p", bufs=6) as w1_pool,
            tc.tile_pool(name="ps0", bufs=4, space="PSUM") as ps0_pool,
            tc.tile_pool(name="ps1", bufs=4, space="PSUM") as ps1_pool,
        ):
            # ---------------- arch-weight prep (tiny, Sync HWDGE first) -----
            # E9[e*3+r] = exp(ae[e] + am[r]); G = (E9 @ K2), col 51 = normalizer
            arch_sb = const.tile([9, 54], F32, tag="arch_sb")
            nc.sync.dma_start(arch_sb[:], arch_d[:, :])
            k2_sb = arch_sb[:, 2:54]

            # first two xT chunk loads issue before exp on the Scalar HWDGE
            # queue (exp's wait would block the in-order queue behind it)
            xt_sb = []
            xfs = []
            for k in range(D // P):
                xf = xf_pool.tile([P, T], F32, tag=f"xf{k}", name=f"xf{k}", bufs=1)
                t = const.tile([P, T], BF16, tag=f"xt{k}", name=f"xt{k}")
                if k < 2:
                    nc.scalar.dma_start(xf[:], xT_d[k * P : (k + 1) * P, :])
                xfs.append(xf)
                xt_sb.append(t)

            v9 = const.tile([9, 1], F32, tag="v9")
            nc.vector.tensor_tensor(v9[:], arch_sb[:, 0:1], arch_sb[:, 1:2], ALU.add)
            e9 = const.tile([9, 1], F32, tag="e9")
            nc.scalar.activation(e9[:], v9[:], AF.Exp)

            g_ps = ps1_pool.tile([P, T], F32, tag="ps1", name="g_ps")[0:1, 0:52]
            nc.tensor.matmul(g_ps[:], e9[:], k2_sb[:], start=True, stop=True)
            g_row = const.tile([1, 52], F32, tag="g_row")
            nc.vector.tensor_copy(g_row[:], g_ps[:])
            # broadcast to all 128 partitions via a k=1 ones-matmul
            ones128 = const.tile([1, P], F32, tag="ones128")
            nc.vector.memset(ones128[:], 1.0)
            gbu_ps = ps1_pool.tile([P, T], F32, tag="ps1", name="gbu_ps")[:, 0:52]
            nc.tensor.matmul(gbu_ps[:], ones128[:], g_row[:], start=True, stop=True)
            rec = const.tile([P, 1], F32, tag="rec")
            nc.vector.reciprocal(rec[:], gbu_ps[:, 51:52])
            gb = const.tile([P, 51], F32, tag="gb")
            nc.vector.tensor_scalar(gb[:], gbu_ps[:, 0:51], rec[:, 0:1], None, ALU.mult)

            # first two casts on DVE (needed by the first k-major steps)
            for k in range(2):
                nc.vector.tensor_copy(xt_sb[k][:], xfs[k][:])
            # remaining chunks: DMA after exp (queue unblocked), cast on Scalar
            for k in range(2, D // P):
                nc.scalar.dma_start(xfs[k][:], xT_d[k * P : (k + 1) * P, :])
            for k in range(2, D // P):
                nc.scalar.activation(xt_sb[k][:], xfs[k][:], AF.Copy)

            # ---------------- effective biases ----------------
            b0_sb = const.tile([P, H // P], F32, tag="b0_sb")
            nc.scalar.dma_start(b0_sb[:], b0r_d[:, :])
            bb0 = const.tile([P, H // P], F32, tag="bb0")
            nc.vector.tensor_tensor(
                bb0[:].rearrange("p (s i) -> p s i", i=2),
                b0_sb[:].rearrange("p (s i) -> p s i", i=2),
                gb[:, 0:16].unsqueeze(2).to_broadcast((P, 16, 2)),
                ALU.mult,
            )
            b1_sb = const.tile([P, D // P], F32, tag="b1_sb")
            nc.scalar.dma_start(b1_sb[:], b1r_d[:, :])
            bb1 = const.tile([P, D // P], F32, tag="bb1")
            for j, (c0, c1) in enumerate([(0, 4), (4, 6), (6, 8)]):
                nc.vector.tensor_scalar(
                    bb1[:, c0:c1], b1_sb[:, c0:c1],
                    gb[:, 48 + j : 49 + j], None, ALU.mult,
                )

            # ---------------- PE warmup ----------------
            # Keep the PE busy from ~t+7us so the HAM clock gate reaches
            # K=8/8 before the first real matmuls; sized to end when the
            # first masked w0 tile is ready (~10.5us).
            junk_w = const.tile([P, 2 * P], BF16, tag="junk_w")
            nc.vector.memset(junk_w[:], 0.0)
            junk_x = const.tile([P, T], BF16, tag="junk_x")
            nc.vector.memset(junk_x[:], 0.0)
            ps_w = ps0_pool.tile([P, T], F32, tag="ps0", name="ps_w")
            for i in range(NWARM):
                sl = (i % 2) * P
                nc.tensor.matmul(
                    ps_w[:], junk_w[:, sl : sl + P], junk_x[:],
                    start=(i == 0), stop=(i == NWARM - 1),
                )

            # persistent hT and output accumulator
            ht_sb = [
                const.tile([P, T], BF16, tag=f"ht{m}", name=f"ht{m}")
                for m in range(H // P)
            ]
            outacc = [
                const.tile([P, T], F32, tag=f"oa{dt}", name=f"oa{dt}")
                for dt in range(D // P)
            ]

            def emit_fill(n, name):
                """Junk matmuls that keep the PE (and its HAM clock gate) busy
                across a DMA-starved stretch; PSUM from the ps1 ring, which is
                idle until L1(pr0)."""
                if n <= 0:
                    return
                ps_f = ps1_pool.tile([P, T], F32, tag="ps1", name=name)
                for i in range(n):
                    sl = (i % 2) * P
                    nc.tensor.matmul(
                        ps_f[:], junk_w[:, sl : sl + P], junk_x[:],
                        start=(i == 0), stop=(i == n - 1),
                    )

            def emit_l0(pr):
                """Layer 0 for h-groups 2pr, 2pr+1, k-major (4 live chains)."""
                for hg in (2 * pr, 2 * pr + 1):
                    if pr == 0 and hg == 1:
                        emit_fill(FILL1, "fill1")
                    w0m_chunks = []  # per d-chunk k: [P, 512] bf16 masked tile
                    w0f_tiles = []
                    for pk in range(4):  # d-chunk pairs (k = 2*pk, 2*pk+1)
                        w0f = w0f_pool.tile([P, 1024], F32, tag="w0f", name="w0f")
                        w0f_tiles.append(w0f)
                        nc.sync.dma_start(
                            w0f[:].rearrange("p (k h) -> p k h", k=2),
                            w0T_d[
                                2 * pk * P : (2 * pk + 2) * P,
                                hg * 512 : (hg + 1) * 512,
                            ].rearrange("(k p) h -> p k h", k=2),
                        )
                        for c in range(2):  # mask+cast per 128-d chunk
                            cbase = _DBLK[2 * pk + c] * 16 + hg * 2
                            w0m = w0_pool.tile([P, 512], BF16, tag="w0m", name="w0m")
                            nc.vector.tensor_tensor(
                                w0m[:].rearrange("p (s c) -> p s c", c=SEG),
                                w0f[:, c * 512 : (c + 1) * 512].rearrange(
                                    "p (s c) -> p s c", c=SEG
                                ),
                                gb[:, cbase : cbase + 2]
                                .unsqueeze(2)
                                .to_broadcast((P, 2, SEG)),
                                ALU.mult,
                            )
                            w0m_chunks.append(w0m)
                    pss = [
                        ps0_pool.tile([P, T], F32, tag="ps0", name=f"ps0_{hg}_{ht}")
                        for ht in range(4)
                    ]
                    for k in range(D // P):
                        for ht in range(4):
                            nc.tensor.matmul(
                                pss[ht][:],
                                w0m_chunks[k][:, ht * P : (ht + 1) * P],
                                xt_sb[k][:],
                                start=(k == 0),
                                stop=(k == D // P - 1),
                            )
                    for ht in range(4):
                        m = hg * 4 + ht
                        nc.scalar.activation(
                            ht_sb[m][:], pss[ht][:], AF.Relu, bias=bb0[:, m : m + 1]
                        )

            def emit_l1(pr):
                """Layer 1 partial for h-group pair pr (K = 8 x 128)."""
                w1m_tiles = []
                for pj in range(4):
                    hc = pr * 8 + 2 * pj
                    w1f = w1f_pool.tile([P, 2048], F32, tag="w1f", name="w1f")
                    nc.sync.dma_start(
                        w1f[:].rearrange("p (k d) -> p k d", k=2),
                        w1T_d[hc * P : (hc + 2) * P, :].rearrange(
                            "(k p) d -> p k d", k=2
                        ),
                    )
                    seg_h = hc // 2
                    w1m = w1_pool.tile([P, 2048], BF16, tag="w1m", name="w1m")
                    ap3m = w1m[:].rearrange("p (k d) -> p k d", k=2)
                    ap3f = w1f[:].rearrange("p (k d) -> p k d", k=2)
                    # in the DMA-bound front (pr<=1) alternate mask tiles
                    # between Scalar and DVE so production keeps up with the
                    # L1 matmul stream; steady state keeps Scalar only
                    on_dve = pr <= 1 and pj % 2 == 1
                    for jd, (c0, c1) in enumerate([(0, 512), (512, 768), (768, 1024)]):
                        sc = gb[:, jd * 16 + seg_h : jd * 16 + seg_h + 1]
                        if on_dve:
                            nc.vector.tensor_scalar(
                                ap3m[:, :, c0:c1], ap3f[:, :, c0:c1],
                                sc, None, ALU.mult,
                            )
                        else:
                            nc.scalar.activation(
                                ap3m[:, :, c0:c1], ap3f[:, :, c0:c1], AF.Copy,
                                scale=sc,
                            )
                    w1m_tiles.append(w1m)

                def finalize(dt, ps):
                    if pr == 0:
                        nc.vector.tensor_scalar(
                            outacc[dt][:], ps[:], bb1[:, dt : dt + 1], None, ALU.add
                        )
                    elif pr < 3:
                        nc.vector.tensor_tensor(
                            outacc[dt][:], ps[:], outacc[dt][:], ALU.add
                        )
                    else:
                        # final add+store in column halves so the store of the
                        # first half overlaps the add of the second
                        for c0, c1 in ((0, T // 2), (T // 2, T)):
                            nc.vector.tensor_tensor(
                                outacc[dt][:, c0:c1], ps[:, c0:c1],
                                outacc[dt][:, c0:c1], ALU.add,
                            )
                            nc.scalar.dma_start(
                                out_d[dt * P : (dt + 1) * P, c0:c1],
                                outacc[dt][:, c0:c1],
                            )

                if pr < 1:
                    # pair-major over 4-chain halves: pairs consumed in DMA
                    # arrival order so the first chains never wait on the
                    # last w1 tile of the phase (w1 stream still catching up)
                    for half in range(2):
                        dts = range(4 * half, 4 * half + 4)
                        pss = [
                            ps1_pool.tile(
                                [P, T], F32, tag="ps1", name=f"ps1_{pr}_{dt}"
                            )
                            for dt in dts
                        ]
                        for pj in range(4):
                            for j in (2 * pj, 2 * pj + 1):
                                for i, dt in enumerate(dts):
                                    off = (j % 2) * 1024 + dt * P
                                    nc.tensor.matmul(
                                        pss[i][:],
                                        w1m_tiles[pj][:, off : off + P],
                                        ht_sb[pr * 8 + j][:],
                                        start=(j == 0),
                                        stop=(j == 7),
                                    )
                        for i, dt in enumerate(dts):
                            finalize(dt, pss[i])
                else:
                    # dt-major: chains end staggered so the finalize/store
                    # tail pipelines instead of bunching at phase end
                    for dt in range(D // P):
                        ps = ps1_pool.tile([P, T], F32, tag="ps1", name="ps1")
                        for j in range(8):
                            off = (j % 2) * 1024 + dt * P
                            nc.tensor.matmul(
                                ps[:],
                                w1m_tiles[j // 2][:, off : off + P],
                                ht_sb[pr * 8 + j][:],
                                start=(j == 0),
                                stop=(j == 7),
                            )
                        finalize(dt, ps)

            # alternating phases: matches the DMA delivery envelope — the PE's
            # cumulative byte demand tracks ~290 GB/s against ~400 delivered,
            # so no phase outruns the weight stream
            for pr in range(4):
                emit_l0(pr)
                if pr == 0:
                    emit_fill(FILL2, "fill2")
                emit_l1(pr)

    nc.compile()
    return nc


_NC_CACHE: dict[str, bass.Bass] = {}


def _get_nc() -> bass.Bass:
    key = str(NWARM)
    if key not in _NC_CACHE:
        _NC_CACHE[key] = _build_nc()
    return _NC_CACHE[key]


def make_in_maps(x, w0, b0, w1, b1, arch_embed, arch_mlp):
    """Host-side layout prep (pure reshape/transpose/tile, no arithmetic)."""
    w0T = np.ascontiguousarray(w0.T)                       # [D, H]
    w1T = np.ascontiguousarray(w1.T)                       # [H, D]
    b0r = np.ascontiguousarray(b0.reshape(H // P, P).T)    # [P, 32]
    b1r = np.ascontiguousarray(b1.reshape(D // P, P).T)    # [P, 8]
    # packed [ae9 | am9 | K2]: pure repeat/tile/concat layout, no arithmetic
    arch = np.concatenate(
        [
            np.repeat(arch_embed, 3).reshape(9, 1),
            np.tile(arch_mlp, 3).reshape(9, 1),
            _K2,
        ],
        axis=1,
    ).astype(np.float32)
    arch = np.ascontiguousarray(arch)                      # [9, 54]
    x3 = x.reshape(N_CORES, T, D)
    return [
        {
            "xT": np.ascontiguousarray(x3[c].T),           # [D, T]
            "w0T": w0T,
            "w1T": w1T,
            "b0r": b0r,
            "b1r": b1r,
            "arch": arch,
        }
        for c in range(N_CORES)
    ]


def kernel(x, w0, b0, w1, b1, arch_embed, arch_mlp):
    x = np.asarray(x, dtype=np.float32)
    w0 = np.asarray(w0, dtype=np.float32)
    b0 = np.asarray(b0, dtype=np.float32)
    w1 = np.asarray(w1, dtype=np.float32)
    b1 = np.asarray(b1, dtype=np.float32)
    arch_embed = np.asarray(arch_embed, dtype=np.float32)
    arch_mlp = np.asarray(arch_mlp, dtype=np.float32)

    in_maps = make_in_maps(x, w0, b0, w1, b1, arch_embed, arch_mlp)
    nc = _get_nc()
    res = run_bass_kernel_spmd(nc, in_maps, core_ids=list(range(N_CORES)))
    out = np.stack([res.results[c]["outT"].T for c in range(N_CORES)], axis=0)
    return np.ascontiguousarray(out)  # [8, 512, 1024] float32

